# revision 37
# baseline (speedup 1.0000x reference)
"""Chamfer loss on 8 Trainium2 NeuronCores — banded KNN formulation.

pred [8192,3], label [8192,3] fp32 ->
scalar = mean_i min_j ||p_i - l_j|| + mean_j min_i ||p_i - l_j||

Both clouds are sorted by coordinate 0 on the host.  Core k owns sorted
pred rows [1024k, 1024k+1024) and a label SLAB of S=1920 consecutive
sorted labels centered on its rows.  Each 128-row tile t computes ONE
[128, W=1024] d^2 block against slab columns [128t, 128t+W) via an
augmented K=5 fp32r matmul (u_i = [-2x, |x|^2, 1], v_j = [y, 1, |y|^2])
— a banded subset (12.5%) of the full 8192^2 distance matrix.

Per core: tiles 0-4 are reduced on device (row-mins via fused
tensor_scalar accum or fp16 4x passes; column-mins folded in fp16 into
two accumulators accA = tile0's window, accB = tile4's window, written
directly by those tiles' drains so no init pass is needed).  Tiles 5-7
are drained to fp16 and shipped to the host.  Drain engines are spread
over ACT / DVE / Pool(gpsimd) so all three run concurrently.

The host finishes: row/col mins of shipped tiles (uint16-view min on
nonneg fp16), 128-lane reduction of accA/accB, cross-core merge in
sorted-rank space, and a CERTIFIED RESCUE: a banded min is provably
exact when it is smaller than the 1-D coordinate gap to its window
edges; uncertified points (window misses, z-tails, edge cores) are
recomputed exactly in numpy — first against an expanded +-E window,
then, for stragglers, against all points.  Result is exact up to fp16
rounding for ANY input distribution.
"""

import sys

if "/opt/trn_rl_repo" not in sys.path:
    sys.path.insert(0, "/opt/trn_rl_repo")

import numpy as np

import concourse.bacc as bacc
import concourse.mybir as mybir
from concourse import tile
from concourse.bass_utils import run_bass_kernel_spmd

F32 = mybir.dt.float32
F16 = mybir.dt.float16
MIN = mybir.AluOpType.min

N_CORES = 8
N_PTS = 8192
ROWS = N_PTS // N_CORES        # pred rows per core
NT = 8                         # 128-row tiles per core
W = 1024                       # label window per tile
S = 128 * (NT - 1) + W         # slab width per core (1920)
BIG = 3.0e38

KEPT = (0, 1, 2, 3)            # tiles reduced on device
SHIP = (4, 5, 6, 7)            # tiles shipped to host (fp16)
ACC_A_T, ACC_B_T = 0, 3        # kept tiles whose drains seed accA/accB
# drain engine per tile: A=ACT copy, V=DVE tensor_scalar (fused rowmin).
# (Pool/gpsimd can't read PSUM and has no ISA TT/TS — DMA/memset only.)
DRAIN = {0: "V", 1: "V", 2: "A", 3: "V", 4: "A", 5: "A", 6: "A", 7: "A"}
TILE_ORDER = (0, 1, 3, 2, 4, 5, 6, 7)
LAST_KEPT = 2                  # last kept tile in TILE_ORDER

RESCUE_E = 1024                # host rescue expansion (ranks per side)
CERT_INFL = 2e-3               # relative inflation of mins before certifying
CERT_ABS = 1e-3                # absolute inflation (distance units)

# The PE's fp32r mode is lossy (~11-bit operand precision on real TRN2) and
# d^2 = |x|^2+|y|^2-2xy cancels catastrophically near the minima, so the
# augmented operands are built as fp16 HI/LO SPLIT PAIRS instead: each
# scaled coordinate s*x = xh + xl with xh = fp16(s*x), xl = fp16(s*x - xh),
# likewise the norms.  All cross products are exact in fp16*fp16->fp32
# PSUM accumulation, so d^2 comes out exact to ~1e-5 absolute (scaled).
# K = 3 coords * 4 (hh, hl, lh, ll) + 2 + 2 norm lanes = 16.
KDIM = 16
SCALE = 16.0                   # coordinate pre-scale (keeps fp16 residues
                               # out of the subnormal range); d^2 scales
                               # by SCALE^2, host divides at the end.


def build_program():
    nc = bacc.Bacc(
        "TRN2",
        target_bir_lowering=False,
        debug=False,
        enable_asserts=False,
        num_devices=N_CORES,
    )
    # u and v merged in one tensor (single input DMA): cols [0, ROWS) = u,
    # [ROWS, ROWS+S) = v slab
    uv_d = nc.dram_tensor("uv", (KDIM, ROWS + S), F16, kind="ExternalInput")
    acc_d = nc.dram_tensor("acc", (128, 2 * W + NT), F16,
                           kind="ExternalOutput")
    lt_d = nc.dram_tensor("lt", (128, len(SHIP) * W), F16,
                          kind="ExternalOutput")

    wA = (128 * ACC_A_T, 128 * ACC_A_T + W)   # accA slab range
    wB = (128 * ACC_B_T, 128 * ACC_B_T + W)   # accB slab range

    with tile.TileContext(nc) as tc:
        with (
            tc.tile_pool(name="const", bufs=1) as const_pool,
            tc.tile_pool(name="sb", bufs=1) as sb_pool,
            tc.tile_pool(name="mm", bufs=4, space="PSUM") as mm_pool,
        ):
            # issue the input DMA from ACT's HWDGE: the ACT sequencer is idle
            # at t=0 while SP spends ~700ns in the framework preamble
            UV = const_pool.tile([KDIM, ROWS + S], F16)
            nc.scalar.dma_start(UV[:], uv_d.ap())

            def u_ap(t):
                return UV[:, t * 128:(t + 1) * 128]

            def v_ap(c0, c1):
                return UV[:, ROWS + c0:ROWS + c1]

            slots = sb_pool.tile([128, NT], F32)
            nc.gpsimd.memset(slots[:], 0.0)
            # acc2 = accA | accB | fp16 copy of the rowmin slots, so the
            # whole device-side result leaves in ONE DMA
            acc2 = sb_pool.tile([128, 2 * W + NT], F16, name="acc2")

            def accA(c0=0, c1=W):
                return acc2[:, c0:c1]

            def accB(c0=0, c1=W):
                return acc2[:, W + c0:W + c1]
            lt = sb_pool.tile([128, len(SHIP) * W], F16, name="lt")
            scr = {
                t: sb_pool.tile([128, W], F16, name=f"scr{t}")
                for t in KEPT if t not in (ACC_A_T, ACC_B_T)
            }
            rsc = sb_pool.tile([128, W], F16, name="rsc")  # rowmin scratch

            def drain_dst(t):
                if t == ACC_A_T:
                    return accA()
                if t == ACC_B_T:
                    return accB()
                if t in scr:
                    return scr[t][:]
                i = SHIP.index(t)
                return lt[:, i * W:(i + 1) * W]

            # kept tiles first, ships after — the acc DMA then fires
            # mid-kernel while ships still drain; each ship tile gets its
            # own DMA right after its drain so the final lt transfer isn't
            # one big end-of-program blob.  t3 (the accB seed, DVE-drained)
            # goes before t2 (ACT-drained, DVE folds) so the in-order DVE
            # queue finishes all acc work as early as possible.
            for t in TILE_ORDER:
                ps = mm_pool.tile([128, W], F32, tag="mm")
                for q in range(W // 512):
                    nc.tensor.matmul(
                        ps[:, q * 512:(q + 1) * 512],
                        u_ap(t),
                        v_ap(128 * t + q * 512, 128 * t + (q + 1) * 512),
                        start=True, stop=True,
                    )
                eng = DRAIN[t]
                dst = drain_dst(t)
                if eng == "A":
                    nc.scalar.copy(dst, ps[:])
                    if t in KEPT:
                        # fp16 4x rowmin pass on DVE
                        nc.vector.tensor_scalar(
                            out=rsc[:], in0=dst, scalar1=BIG, scalar2=None,
                            op0=MIN, op1=MIN,
                            accum_out=slots[:, t:t + 1],
                        )
                else:
                    e = nc.vector if eng == "V" else nc.gpsimd
                    e.tensor_scalar(
                        out=dst, in0=ps[:], scalar1=BIG, scalar2=None,
                        op0=MIN, op1=MIN,
                        accum_out=slots[:, t:t + 1],
                    )
                # col-min folds for kept tiles not seeding an acc:
                # in-place TT min into the overlapping acc regions.
                if t in scr:
                    fe = nc.vector
                    lo, hi = 128 * t, 128 * t + W
                    # accA part
                    a0, a1 = max(lo, wA[0]), min(hi, wA[1])
                    if a1 > a0:
                        fe.tensor_tensor(
                            out=accA(a0 - wA[0], a1 - wA[0]),
                            in0=accA(a0 - wA[0], a1 - wA[0]),
                            in1=scr[t][:, a0 - lo:a1 - lo], op=MIN,
                        )
                    # accB part (skip overlap already in accA)
                    b0, b1 = max(lo, wB[0], a1), min(hi, wB[1])
                    if b1 > b0:
                        fe.tensor_tensor(
                            out=accB(b0 - wB[0], b1 - wB[0]),
                            in0=accB(b0 - wB[0], b1 - wB[0]),
                            in1=scr[t][:, b0 - lo:b1 - lo], op=MIN,
                        )
                if t == LAST_KEPT:
                    # all folds + slot writes done — pack the fp32 slots into
                    # acc2's tail as fp16 and stream the whole device-side
                    # result out in ONE gpsimd SWDGE DMA (desc-gen on the
                    # idle Pool engine, off the shared HWDGE of the ships)
                    nc.vector.tensor_copy(acc2[:, 2 * W:2 * W + NT], slots[:])
                    nc.gpsimd.dma_start(acc_d.ap(), acc2[:])
                if t in SHIP:
                    i = SHIP.index(t)
                    nc.sync.dma_start(
                        lt_d.ap()[:, i * W:(i + 1) * W],
                        lt[:, i * W:(i + 1) * W],
                    )

    nc.compile()
    return nc


_NC_CACHE = None


def _fp16_nonneg_min(a, axis):
    """min over nonnegative fp16 via uint16 view (negative encodings sort
    above all nonnegative ones, matching the d2 >= 0 clamp)."""
    return a.view(np.uint16).min(axis=axis).view(np.float16)


def _slab_starts():
    off = (W - 128) // 2  # window margin each side of a tile's 128 ranks
    return [min(max(1024 * k - off, 0), N_PTS - S) for k in range(N_CORES)]


def _window_starts():
    """Global label-rank start of tile g's window, g = 8k + t."""
    slab = _slab_starts()
    return np.array([slab[g // 8] + 128 * (g % 8) for g in range(64)])


def _split16(a):
    """fp16 hi/lo split of f64 array: a ~= hi + lo with both fp16."""
    hi = a.astype(np.float16)
    lo = (a - hi.astype(np.float64)).astype(np.float16)
    return hi, lo


def _aug_v(y):
    """[KDIM, n] fp16 augmented label operand (scaled hi/lo split)."""
    ys = y.astype(np.float64).T * SCALE               # [3, n]
    yh, yl = _split16(ys)
    yeff = yh.astype(np.float64) + yl.astype(np.float64)
    n2 = (yeff ** 2).sum(0)
    nh, nl = _split16(n2)
    v = np.zeros((KDIM, y.shape[0]), np.float16)
    for c in range(3):
        v[4 * c + 0] = yh[c]
        v[4 * c + 1] = yl[c]
        v[4 * c + 2] = yh[c]
        v[4 * c + 3] = yl[c]
    v[12] = 1.0
    v[13] = 1.0
    v[14] = nh
    v[15] = nl
    return v


def _aug_u(x):
    """[KDIM, n] fp16 augmented pred operand (scaled hi/lo split)."""
    xs = x.astype(np.float64).T * SCALE
    xh, xl = _split16(xs)
    xeff = xh.astype(np.float64) + xl.astype(np.float64)
    n2 = (xeff ** 2).sum(0)
    nh, nl = _split16(n2)
    u = np.zeros((KDIM, x.shape[0]), np.float16)
    for c in range(3):
        u[4 * c + 0] = -2.0 * xh[c]
        u[4 * c + 1] = -2.0 * xh[c]
        u[4 * c + 2] = -2.0 * xl[c]
        u[4 * c + 3] = -2.0 * xl[c]
    u[12] = nh
    u[13] = nl
    u[14] = 1.0
    u[15] = 1.0
    return u


def _run(pred: np.ndarray, label: np.ndarray, trace: bool = False):
    global _NC_CACHE
    if _NC_CACHE is None:
        _NC_CACHE = build_program()
    nc = _NC_CACHE

    pred = np.ascontiguousarray(pred, dtype=np.float32)
    label = np.ascontiguousarray(label, dtype=np.float32)

    po = np.argsort(pred[:, 0], kind="stable")
    lo_ = np.argsort(label[:, 0], kind="stable")
    P = pred[po]
    L = label[lo_]
    zP = P[:, 0].astype(np.float64)
    zL = L[:, 0].astype(np.float64)

    slab = _slab_starts()
    vfull = _aug_v(L)
    ufull = _aug_u(P)
    in_maps = []
    for k in range(N_CORES):
        uv = np.empty((KDIM, ROWS + S), np.float16)
        uv[:, 0:ROWS] = ufull[:, k * ROWS:(k + 1) * ROWS]
        uv[:, ROWS:] = vfull[:, slab[k]:slab[k] + S]
        in_maps.append({"uv": uv})

    last_err = None
    for attempt in range(3):
        try:
            res = run_bass_kernel_spmd(
                nc, in_maps, core_ids=list(range(N_CORES)), trace=trace
            )
            break
        except Exception as e:  # noqa: BLE001
            last_err = e
            import time as _time

            _time.sleep(2.0 * (attempt + 1))
    else:
        raise last_err

    o_g = _window_starts()
    rmin = np.empty(N_PTS)                 # d^2 row mins, sorted pred order
    cmin = np.full(N_PTS, np.inf)          # d^2 col mins, sorted label order
    for k in range(N_CORES):
        r = res.results[k]
        acc = r["acc"]
        sl = np.clip(acc[:, 2 * W:2 * W + NT].astype(np.float64), 0.0, None)
        for t in KEPT:
            g = 8 * k + t
            rmin[128 * g:128 * g + 128] = sl[:, t]
        colA = _fp16_nonneg_min(acc[:, 0:W], 0).astype(np.float64)
        colB = _fp16_nonneg_min(acc[:, W:2 * W], 0).astype(np.float64)
        a0 = slab[k] + 128 * ACC_A_T
        b0 = slab[k] + 128 * ACC_B_T
        cmin[a0:a0 + W] = np.minimum(cmin[a0:a0 + W], np.clip(colA, 0, None))
        cmin[b0:b0 + W] = np.minimum(cmin[b0:b0 + W], np.clip(colB, 0, None))
        lt = r["lt"]
        for i, t in enumerate(SHIP):
            g = 8 * k + t
            blk = lt[:, i * W:(i + 1) * W]
            rmin[128 * g:128 * g + 128] = np.clip(
                _fp16_nonneg_min(blk, 1).astype(np.float64), 0, None)
            cblk = np.clip(_fp16_nonneg_min(blk, 0).astype(np.float64), 0, None)
            o = o_g[g]
            cmin[o:o + W] = np.minimum(cmin[o:o + W], cblk)

    rmin = np.sqrt(rmin) / SCALE
    cmin = np.sqrt(cmin) / SCALE

    # ---- certified rescue (host, exact) --------------------------------
    P64 = P.astype(np.float64)
    L64 = L.astype(np.float64)

    def row_exact(idx, lo_r, hi_r):
        """exact d over label ranks [lo_r, hi_r) for pred rows idx."""
        d2 = ((P64[idx][:, None, :] - L64[lo_r:hi_r][None, :, :]) ** 2).sum(-1)
        return np.sqrt(d2.min(1))

    def col_exact(idx, lo_r, hi_r):
        d2 = ((P64[lo_r:hi_r][:, None, :] - L64[idx][None, :, :]) ** 2).sum(-1)
        return np.sqrt(d2.min(0))

    # row side: window [wlo, whi) per sorted pred rank
    wlo = o_g[np.arange(N_PTS) // 128]
    whi = wlo + W

    def row_unc_mask(vals, lo_b, hi_b):
        infl = vals * (1 + CERT_INFL) + CERT_ABS
        okl = (lo_b == 0) | (infl <= zP - zL[np.maximum(lo_b - 1, 0)])
        okr = (hi_b == N_PTS) | (infl <= zL[np.minimum(hi_b, N_PTS - 1)] - zP)
        return ~(okl & okr)

    unc = row_unc_mask(rmin, wlo, whi)
    if unc.any():
        idx = np.nonzero(unc)[0]
        elo = np.maximum(wlo[idx] - RESCUE_E, 0)
        ehi = np.minimum(whi[idx] + RESCUE_E, N_PTS)
        # batch by equal-ish ranges: just loop python over idx in chunks of
        # identical (elo, ehi) is overkill — vectorize per point via padded
        # gather is messy; a direct loop over a few thousand rows of <=3k
        # labels is fast enough in numpy if batched per 128-tile.
        for g in np.unique(idx // 128):
            sel = idx[idx // 128 == g]
            lo_r, hi_r = int(elo[idx // 128 == g][0]), int(ehi[idx // 128 == g][0])
            rmin[sel] = np.minimum(rmin[sel], row_exact(sel, lo_r, hi_r))
        # re-certify against expanded window; stragglers get full rows
        elo_f = np.maximum(wlo - RESCUE_E, 0)
        ehi_f = np.minimum(whi + RESCUE_E, N_PTS)
        unc2 = row_unc_mask(rmin, elo_f, ehi_f) & unc
        if unc2.any():
            sel = np.nonzero(unc2)[0]
            rmin[sel] = row_exact(sel, 0, N_PTS)

    # col side: contiguous covered pred-rank run around nearest pred rank
    covered = np.zeros((64, N_PTS), bool)
    for g in range(64):
        covered[g, o_g[g]:o_g[g] + W] = True
    rstar = np.clip(np.searchsorted(zP, zL), 0, N_PTS - 1)
    gstar = rstar // 128
    jj = np.arange(N_PTS)
    # walk down/up from gstar while covered (vectorized over 64 steps)
    glo = gstar.copy()
    for _ in range(64):
        can = (glo > 0) & covered[np.maximum(glo - 1, 0), jj]
        if not can.any():
            break
        glo = np.where(can, glo - 1, glo)
    ghi = gstar.copy()
    for _ in range(64):
        can = (ghi < 63) & covered[np.minimum(ghi + 1, 63), jj]
        if not can.any():
            break
        ghi = np.where(can, ghi + 1, ghi)
    ok_center = covered[gstar, jj]
    clo = 128 * glo
    chi = 128 * ghi + 128

    def col_unc_mask(vals, lo_b, hi_b, okc):
        infl = vals * (1 + CERT_INFL) + CERT_ABS
        okl = (lo_b == 0) | (infl <= zL - zP[np.maximum(lo_b - 1, 0)])
        okr = (hi_b == N_PTS) | (infl <= zP[np.minimum(hi_b, N_PTS - 1)] - zL)
        return ~(okl & okr & okc)

    cunc = col_unc_mask(cmin, clo, chi, ok_center)
    if cunc.any():
        idx = np.nonzero(cunc)[0]
        elo = np.maximum(clo[idx] - RESCUE_E, 0)
        ehi = np.minimum(chi[idx] + RESCUE_E, N_PTS)
        for b in np.unique(idx // 256):
            m = idx // 256 == b
            sel = idx[m]
            lo_r, hi_r = int(elo[m].min()), int(ehi[m].max())
            cmin[sel] = np.minimum(cmin[sel], col_exact(sel, lo_r, hi_r))
        elo_f = np.maximum(clo - RESCUE_E, 0)
        ehi_f = np.minimum(chi + RESCUE_E, N_PTS)
        cunc2 = col_unc_mask(cmin, elo_f, ehi_f, np.ones(N_PTS, bool)) & cunc
        if cunc2.any():
            sel = np.nonzero(cunc2)[0]
            cmin[sel] = col_exact(sel, 0, N_PTS)

    out = rmin.mean() + cmin.mean()
    return np.float32(out), res


def kernel(pred: np.ndarray, label: np.ndarray) -> np.ndarray:
    return _run(pred, label)[0]


# revision 39
# speedup vs baseline: 1.0770x; 1.0770x over previous
"""Chamfer loss on 8 Trainium2 NeuronCores — banded KNN formulation.

pred [8192,3], label [8192,3] fp32 ->
scalar = mean_i min_j ||p_i - l_j|| + mean_j min_i ||p_i - l_j||

Both clouds are sorted by coordinate 0 on the host.  Core k owns sorted
pred rows [1024k, 1024k+1024) and a label SLAB of S=1920 consecutive
sorted labels centered on its rows.  Each 128-row tile t computes ONE
[128, W=1024] d^2 block against slab columns [128t, 128t+W) via an
augmented K=5 fp32r matmul (u_i = [-2x, |x|^2, 1], v_j = [y, 1, |y|^2])
— a banded subset (12.5%) of the full 8192^2 distance matrix.

Per core: tiles 0-4 are reduced on device (row-mins via fused
tensor_scalar accum or fp16 4x passes; column-mins folded in fp16 into
two accumulators accA = tile0's window, accB = tile4's window, written
directly by those tiles' drains so no init pass is needed).  Tiles 5-7
are drained to fp16 and shipped to the host.  Drain engines are spread
over ACT / DVE / Pool(gpsimd) so all three run concurrently.

The host finishes: row/col mins of shipped tiles (uint16-view min on
nonneg fp16), 128-lane reduction of accA/accB, cross-core merge in
sorted-rank space, and a CERTIFIED RESCUE: a banded min is provably
exact when it is smaller than the 1-D coordinate gap to its window
edges; uncertified points (window misses, z-tails, edge cores) are
recomputed exactly in numpy — first against an expanded +-E window,
then, for stragglers, against all points.  Result is exact up to fp16
rounding for ANY input distribution.
"""

import sys

if "/opt/trn_rl_repo" not in sys.path:
    sys.path.insert(0, "/opt/trn_rl_repo")

import numpy as np

import concourse.bacc as bacc
import concourse.mybir as mybir
from concourse import tile
from concourse.bass_utils import run_bass_kernel_spmd

F32 = mybir.dt.float32
F16 = mybir.dt.float16
MIN = mybir.AluOpType.min

N_CORES = 8
N_PTS = 8192
ROWS = N_PTS // N_CORES        # pred rows per core
NT = 8                         # 128-row tiles per core
W = 1024                       # label window per tile
S = 128 * (NT - 1) + W         # slab width per core (1920)
BIG = 3.0e38

KEPT = (0, 1, 2, 3)            # tiles reduced on device
SHIP = (4, 5, 6, 7)            # tiles shipped to host (fp16)
ACC_A_T, ACC_B_T = 0, 3        # kept tiles whose drains seed accA/accB
# drain engine per tile: A=ACT copy, V=DVE tensor_scalar (fused rowmin).
# (Pool/gpsimd can't read PSUM and has no ISA TT/TS — DMA/memset only.)
DRAIN = {0: "V", 1: "V", 2: "A", 3: "V", 4: "A", 5: "A", 6: "A", 7: "A"}
TILE_ORDER = (0, 1, 2, 3, 4, 5, 6, 7)
LAST_KEPT = 3                  # last kept tile in TILE_ORDER

RESCUE_E = 1024                # host rescue expansion (ranks per side)
CERT_INFL = 2e-3               # relative inflation of mins before certifying
CERT_ABS = 1e-3                # absolute inflation (distance units)

# The PE's fp32r mode is lossy (~11-bit operand precision on real TRN2) and
# d^2 = |x|^2+|y|^2-2xy cancels catastrophically near the minima, so the
# augmented operands are built as fp16 HI/LO SPLIT PAIRS instead: each
# scaled coordinate s*x = xh + xl with xh = fp16(s*x), xl = fp16(s*x - xh),
# likewise the norms.  All cross products are exact in fp16*fp16->fp32
# PSUM accumulation, so d^2 comes out exact to ~1e-5 absolute (scaled).
# K = 3 coords * 4 (hh, hl, lh, ll) + 2 + 2 norm lanes = 16.
KDIM = 16
SCALE = 16.0                   # coordinate pre-scale (keeps fp16 residues
                               # out of the subnormal range); d^2 scales
                               # by SCALE^2, host divides at the end.


def build_program():
    nc = bacc.Bacc(
        "TRN2",
        target_bir_lowering=False,
        debug=False,
        enable_asserts=False,
        num_devices=N_CORES,
    )
    # u and v merged in one tensor (single input DMA): cols [0, ROWS) = u,
    # [ROWS, ROWS+S) = v slab
    uv_d = nc.dram_tensor("uv", (KDIM, ROWS + S), F16, kind="ExternalInput")
    acc_d = nc.dram_tensor("acc", (128, 2 * W + NT), F16,
                           kind="ExternalOutput")
    lt_d = nc.dram_tensor("lt", (128, len(SHIP) * W), F16,
                          kind="ExternalOutput")

    wA = (128 * ACC_A_T, 128 * ACC_A_T + W)   # accA slab range
    wB = (128 * ACC_B_T, 128 * ACC_B_T + W)   # accB slab range

    with tile.TileContext(nc) as tc:
        with (
            tc.tile_pool(name="const", bufs=1) as const_pool,
            tc.tile_pool(name="sb", bufs=1) as sb_pool,
            tc.tile_pool(name="mm", bufs=4, space="PSUM") as mm_pool,
        ):
            # issue the input DMA from ACT's HWDGE: the ACT sequencer is idle
            # at t=0 while SP spends ~700ns in the framework preamble
            UV = const_pool.tile([KDIM, ROWS + S], F16)
            nc.scalar.dma_start(UV[:], uv_d.ap())

            def u_ap(t):
                return UV[:, t * 128:(t + 1) * 128]

            def v_ap(c0, c1):
                return UV[:, ROWS + c0:ROWS + c1]

            slots = sb_pool.tile([128, NT], F32)
            nc.gpsimd.memset(slots[:], 0.0)
            # acc2 = accA | accB | fp16 copy of the rowmin slots, so the
            # whole device-side result leaves in ONE DMA
            acc2 = sb_pool.tile([128, 2 * W + NT], F16, name="acc2")

            def accA(c0=0, c1=W):
                return acc2[:, c0:c1]

            def accB(c0=0, c1=W):
                return acc2[:, W + c0:W + c1]
            lt = sb_pool.tile([128, len(SHIP) * W], F16, name="lt")
            scr = {
                t: sb_pool.tile([128, W], F16, name=f"scr{t}")
                for t in KEPT if t not in (ACC_A_T, ACC_B_T)
            }
            rsc = sb_pool.tile([128, W], F16, name="rsc")  # rowmin scratch

            def drain_dst(t):
                if t == ACC_A_T:
                    return accA()
                if t == ACC_B_T:
                    return accB()
                if t in scr:
                    return scr[t][:]
                i = SHIP.index(t)
                return lt[:, i * W:(i + 1) * W]

            # kept tiles first, ships after — the acc DMA then fires
            # mid-kernel while ships still drain; each ship tile gets its
            # own DMA right after its drain so the final lt transfer isn't
            # one big end-of-program blob.  t3 (the accB seed, DVE-drained)
            # goes before t2 (ACT-drained, DVE folds) so the in-order DVE
            # queue finishes all acc work as early as possible.
            for t in TILE_ORDER:
                ps = mm_pool.tile([128, W], F32, tag="mm")
                for q in range(W // 512):
                    nc.tensor.matmul(
                        ps[:, q * 512:(q + 1) * 512],
                        u_ap(t),
                        v_ap(128 * t + q * 512, 128 * t + (q + 1) * 512),
                        start=True, stop=True,
                    )
                eng = DRAIN[t]
                dst = drain_dst(t)
                if eng == "A":
                    nc.scalar.copy(dst, ps[:])
                    if t in KEPT:
                        # fp16 4x rowmin pass on DVE
                        nc.vector.tensor_scalar(
                            out=rsc[:], in0=dst, scalar1=BIG, scalar2=None,
                            op0=MIN, op1=MIN,
                            accum_out=slots[:, t:t + 1],
                        )
                else:
                    e = nc.vector if eng == "V" else nc.gpsimd
                    e.tensor_scalar(
                        out=dst, in0=ps[:], scalar1=BIG, scalar2=None,
                        op0=MIN, op1=MIN,
                        accum_out=slots[:, t:t + 1],
                    )
                # col-min folds for kept tiles not seeding an acc:
                # in-place TT min into the overlapping acc regions.
                if t in scr:
                    fe = nc.vector
                    lo, hi = 128 * t, 128 * t + W
                    # accA part
                    a0, a1 = max(lo, wA[0]), min(hi, wA[1])
                    if a1 > a0:
                        fe.tensor_tensor(
                            out=accA(a0 - wA[0], a1 - wA[0]),
                            in0=accA(a0 - wA[0], a1 - wA[0]),
                            in1=scr[t][:, a0 - lo:a1 - lo], op=MIN,
                        )
                    # accB part (skip overlap already in accA)
                    b0, b1 = max(lo, wB[0], a1), min(hi, wB[1])
                    if b1 > b0:
                        fe.tensor_tensor(
                            out=accB(b0 - wB[0], b1 - wB[0]),
                            in0=accB(b0 - wB[0], b1 - wB[0]),
                            in1=scr[t][:, b0 - lo:b1 - lo], op=MIN,
                        )
                if t == LAST_KEPT:
                    # all folds + slot writes done — pack the fp32 slots
                    # into acc2's tail as fp16 and stream the device-side
                    # results out via gpsimd SWDGE (desc-gen on the idle
                    # Pool engine, off the shared HWDGE of the ships).
                    # accA's region is final before accB's (t3 drain), so
                    # two DMAs let accA's transfer clear the bus early.
                    nc.vector.tensor_copy(acc2[:, 2 * W:2 * W + NT], slots[:])
                    nc.gpsimd.dma_start(acc_d.ap()[:, 0:W], acc2[:, 0:W])
                    nc.gpsimd.dma_start(
                        acc_d.ap()[:, W:2 * W + NT], acc2[:, W:2 * W + NT]
                    )
                if t in SHIP:
                    i = SHIP.index(t)
                    nc.sync.dma_start(
                        lt_d.ap()[:, i * W:(i + 1) * W],
                        lt[:, i * W:(i + 1) * W],
                    )

    nc.compile()
    return nc


_NC_CACHE = None


def _fp16_nonneg_min(a, axis):
    """min over nonnegative fp16 via uint16 view (negative encodings sort
    above all nonnegative ones, matching the d2 >= 0 clamp)."""
    return a.view(np.uint16).min(axis=axis).view(np.float16)


def _slab_starts():
    off = (W - 128) // 2  # window margin each side of a tile's 128 ranks
    return [min(max(1024 * k - off, 0), N_PTS - S) for k in range(N_CORES)]


def _window_starts():
    """Global label-rank start of tile g's window, g = 8k + t."""
    slab = _slab_starts()
    return np.array([slab[g // 8] + 128 * (g % 8) for g in range(64)])


def _split16(a):
    """fp16 hi/lo split of f64 array: a ~= hi + lo with both fp16."""
    hi = a.astype(np.float16)
    lo = (a - hi.astype(np.float64)).astype(np.float16)
    return hi, lo


def _aug_v(y):
    """[KDIM, n] fp16 augmented label operand (scaled hi/lo split)."""
    ys = y.astype(np.float64).T * SCALE               # [3, n]
    yh, yl = _split16(ys)
    yeff = yh.astype(np.float64) + yl.astype(np.float64)
    n2 = (yeff ** 2).sum(0)
    nh, nl = _split16(n2)
    v = np.zeros((KDIM, y.shape[0]), np.float16)
    for c in range(3):
        v[4 * c + 0] = yh[c]
        v[4 * c + 1] = yl[c]
        v[4 * c + 2] = yh[c]
        v[4 * c + 3] = yl[c]
    v[12] = 1.0
    v[13] = 1.0
    v[14] = nh
    v[15] = nl
    return v


def _aug_u(x):
    """[KDIM, n] fp16 augmented pred operand (scaled hi/lo split)."""
    xs = x.astype(np.float64).T * SCALE
    xh, xl = _split16(xs)
    xeff = xh.astype(np.float64) + xl.astype(np.float64)
    n2 = (xeff ** 2).sum(0)
    nh, nl = _split16(n2)
    u = np.zeros((KDIM, x.shape[0]), np.float16)
    for c in range(3):
        u[4 * c + 0] = -2.0 * xh[c]
        u[4 * c + 1] = -2.0 * xh[c]
        u[4 * c + 2] = -2.0 * xl[c]
        u[4 * c + 3] = -2.0 * xl[c]
    u[12] = nh
    u[13] = nl
    u[14] = 1.0
    u[15] = 1.0
    return u


def _run(pred: np.ndarray, label: np.ndarray, trace: bool = False):
    global _NC_CACHE
    if _NC_CACHE is None:
        _NC_CACHE = build_program()
    nc = _NC_CACHE

    pred = np.ascontiguousarray(pred, dtype=np.float32)
    label = np.ascontiguousarray(label, dtype=np.float32)

    po = np.argsort(pred[:, 0], kind="stable")
    lo_ = np.argsort(label[:, 0], kind="stable")
    P = pred[po]
    L = label[lo_]
    zP = P[:, 0].astype(np.float64)
    zL = L[:, 0].astype(np.float64)

    slab = _slab_starts()
    vfull = _aug_v(L)
    ufull = _aug_u(P)
    in_maps = []
    for k in range(N_CORES):
        uv = np.empty((KDIM, ROWS + S), np.float16)
        uv[:, 0:ROWS] = ufull[:, k * ROWS:(k + 1) * ROWS]
        uv[:, ROWS:] = vfull[:, slab[k]:slab[k] + S]
        in_maps.append({"uv": uv})

    last_err = None
    for attempt in range(3):
        try:
            res = run_bass_kernel_spmd(
                nc, in_maps, core_ids=list(range(N_CORES)), trace=trace
            )
            break
        except Exception as e:  # noqa: BLE001
            last_err = e
            import time as _time

            _time.sleep(2.0 * (attempt + 1))
    else:
        raise last_err

    o_g = _window_starts()
    rmin = np.empty(N_PTS)                 # d^2 row mins, sorted pred order
    cmin = np.full(N_PTS, np.inf)          # d^2 col mins, sorted label order
    for k in range(N_CORES):
        r = res.results[k]
        acc = r["acc"]
        sl = np.clip(acc[:, 2 * W:2 * W + NT].astype(np.float64), 0.0, None)
        for t in KEPT:
            g = 8 * k + t
            rmin[128 * g:128 * g + 128] = sl[:, t]
        colA = _fp16_nonneg_min(acc[:, 0:W], 0).astype(np.float64)
        colB = _fp16_nonneg_min(acc[:, W:2 * W], 0).astype(np.float64)
        a0 = slab[k] + 128 * ACC_A_T
        b0 = slab[k] + 128 * ACC_B_T
        cmin[a0:a0 + W] = np.minimum(cmin[a0:a0 + W], np.clip(colA, 0, None))
        cmin[b0:b0 + W] = np.minimum(cmin[b0:b0 + W], np.clip(colB, 0, None))
        lt = r["lt"]
        for i, t in enumerate(SHIP):
            g = 8 * k + t
            blk = lt[:, i * W:(i + 1) * W]
            rmin[128 * g:128 * g + 128] = np.clip(
                _fp16_nonneg_min(blk, 1).astype(np.float64), 0, None)
            cblk = np.clip(_fp16_nonneg_min(blk, 0).astype(np.float64), 0, None)
            o = o_g[g]
            cmin[o:o + W] = np.minimum(cmin[o:o + W], cblk)

    rmin = np.sqrt(rmin) / SCALE
    cmin = np.sqrt(cmin) / SCALE

    # ---- certified rescue (host, exact) --------------------------------
    P64 = P.astype(np.float64)
    L64 = L.astype(np.float64)

    def row_exact(idx, lo_r, hi_r):
        """exact d over label ranks [lo_r, hi_r) for pred rows idx."""
        d2 = ((P64[idx][:, None, :] - L64[lo_r:hi_r][None, :, :]) ** 2).sum(-1)
        return np.sqrt(d2.min(1))

    def col_exact(idx, lo_r, hi_r):
        d2 = ((P64[lo_r:hi_r][:, None, :] - L64[idx][None, :, :]) ** 2).sum(-1)
        return np.sqrt(d2.min(0))

    # row side: window [wlo, whi) per sorted pred rank
    wlo = o_g[np.arange(N_PTS) // 128]
    whi = wlo + W

    def row_unc_mask(vals, lo_b, hi_b):
        infl = vals * (1 + CERT_INFL) + CERT_ABS
        okl = (lo_b == 0) | (infl <= zP - zL[np.maximum(lo_b - 1, 0)])
        okr = (hi_b == N_PTS) | (infl <= zL[np.minimum(hi_b, N_PTS - 1)] - zP)
        return ~(okl & okr)

    unc = row_unc_mask(rmin, wlo, whi)
    if unc.any():
        idx = np.nonzero(unc)[0]
        elo = np.maximum(wlo[idx] - RESCUE_E, 0)
        ehi = np.minimum(whi[idx] + RESCUE_E, N_PTS)
        # batch by equal-ish ranges: just loop python over idx in chunks of
        # identical (elo, ehi) is overkill — vectorize per point via padded
        # gather is messy; a direct loop over a few thousand rows of <=3k
        # labels is fast enough in numpy if batched per 128-tile.
        for g in np.unique(idx // 128):
            sel = idx[idx // 128 == g]
            lo_r, hi_r = int(elo[idx // 128 == g][0]), int(ehi[idx // 128 == g][0])
            rmin[sel] = np.minimum(rmin[sel], row_exact(sel, lo_r, hi_r))
        # re-certify against expanded window; stragglers get full rows
        elo_f = np.maximum(wlo - RESCUE_E, 0)
        ehi_f = np.minimum(whi + RESCUE_E, N_PTS)
        unc2 = row_unc_mask(rmin, elo_f, ehi_f) & unc
        if unc2.any():
            sel = np.nonzero(unc2)[0]
            rmin[sel] = row_exact(sel, 0, N_PTS)

    # col side: contiguous covered pred-rank run around nearest pred rank
    covered = np.zeros((64, N_PTS), bool)
    for g in range(64):
        covered[g, o_g[g]:o_g[g] + W] = True
    rstar = np.clip(np.searchsorted(zP, zL), 0, N_PTS - 1)
    gstar = rstar // 128
    jj = np.arange(N_PTS)
    # walk down/up from gstar while covered (vectorized over 64 steps)
    glo = gstar.copy()
    for _ in range(64):
        can = (glo > 0) & covered[np.maximum(glo - 1, 0), jj]
        if not can.any():
            break
        glo = np.where(can, glo - 1, glo)
    ghi = gstar.copy()
    for _ in range(64):
        can = (ghi < 63) & covered[np.minimum(ghi + 1, 63), jj]
        if not can.any():
            break
        ghi = np.where(can, ghi + 1, ghi)
    ok_center = covered[gstar, jj]
    clo = 128 * glo
    chi = 128 * ghi + 128

    def col_unc_mask(vals, lo_b, hi_b, okc):
        infl = vals * (1 + CERT_INFL) + CERT_ABS
        okl = (lo_b == 0) | (infl <= zL - zP[np.maximum(lo_b - 1, 0)])
        okr = (hi_b == N_PTS) | (infl <= zP[np.minimum(hi_b, N_PTS - 1)] - zL)
        return ~(okl & okr & okc)

    cunc = col_unc_mask(cmin, clo, chi, ok_center)
    if cunc.any():
        idx = np.nonzero(cunc)[0]
        elo = np.maximum(clo[idx] - RESCUE_E, 0)
        ehi = np.minimum(chi[idx] + RESCUE_E, N_PTS)
        for b in np.unique(idx // 256):
            m = idx // 256 == b
            sel = idx[m]
            lo_r, hi_r = int(elo[m].min()), int(ehi[m].max())
            cmin[sel] = np.minimum(cmin[sel], col_exact(sel, lo_r, hi_r))
        elo_f = np.maximum(clo - RESCUE_E, 0)
        ehi_f = np.minimum(chi + RESCUE_E, N_PTS)
        cunc2 = col_unc_mask(cmin, elo_f, ehi_f, np.ones(N_PTS, bool)) & cunc
        if cunc2.any():
            sel = np.nonzero(cunc2)[0]
            cmin[sel] = col_exact(sel, 0, N_PTS)

    out = rmin.mean() + cmin.mean()
    return np.float32(out), res


def kernel(pred: np.ndarray, label: np.ndarray) -> np.ndarray:
    return _run(pred, label)[0]
